# revision 4
# baseline (speedup 1.0000x reference)
"""Causal depthwise conv1d (B=8, S=4096, H=2048, KS=4) on 8 trn2 NeuronCores.

Strategy (v3 — uint8 output wire, 3-way engine balance):
  - Shard batch across the 8 cores (one batch element each, no halo needed).
  - bf16 x on the wire (host casts + transposes to (H, S)): 16 MiB in/core.
  - uint8 y on the wire: the kernel emits u8 = rne(y*r_c + 128) with a
    per-channel scale r_c = 126.5 / (sum_k |w_k,c| * max_s |x_c,s| + |b_c|)
    computed on the host and folded into every device-side weight. The host
    dequantizes y = (u8 - 128)/r_c. 8 MiB out/core instead of 16. Measured
    end-to-end max-rel-err ~7e-3 vs the 2e-2 gate (deterministic inputs).
  - Per half-block (128 ch x 2048 cols), two variants batching the work:
    A-half (most):
      PE  : taps w0,w1,w2 as diag matmuls -> PSUM        12 MM ~2.62us
      ACT : e = ps + (b*r+128) -> fp16 (0.125 quantum)   ~1.97us
      DVE : m = x*w3r + e -> fp16 (stt, 2x mode)         ~1.27us
            y = u8(m)      (ts, 2x_2P mode)              ~1.13us
    B-half (every B_PERIOD-th, rebalances PE->ACT):
      PE  : taps w1,w2 only                              8 MM ~1.75us
      DVE : t03 = x(-3)*w0r + e -> fp16 ; m = x*w3r+t03  ~2.54us
      ACT : e as above PLUS y = u8(m) (convert emitted
            one half late to avoid FIFO head-block)      ~3.89us
    Totals ~ PE 79 / DVE 78 / ACT 75 / DMA ~70us.
  - Fill fix: block-0 diag weights live in a tiny separate dram tensor
    (98KB) so the first LDWEIGHTS doesn't wait for the full 1.5MB table.
  - Ring hygiene (from the bf16 baseline): x loads on the sync ring,
    stores + PAD memsets on gpsimd SWDGE, scalar ring = ACT only.
"""

import numpy as np

B, S, H, KS = 8, 4096, 2048, 4
NCORES = 8
PB = 128            # SBUF partitions
HB = H // PB        # 16 channel blocks per core
PAD = 4             # left zero-pad columns in the x tile (3 used + 1 align)
HW_ = 2048          # half-block width (PSUM tile = 4 banks)
BANK = 512          # PSUM bank width in f32 elements
NPE = 3             # taps held on PE (w0, w1, w2); w3 via the DVE stt

OFFSET = 128.0      # HW converts with round-to-nearest-even + saturate
B_PERIOD = 5        # every Nth half-block is a B-half (0 = none)
B_PHASE = 2         # gidx % B_PERIOD == B_PHASE -> B-half

RUN_KWARGS = {}
LAST_RESULTS = []

_cached = {}


def _build():
    import concourse.bacc as bacc
    import concourse.mybir as mybir
    import concourse.tile as tile

    f32 = mybir.dt.float32
    bf16 = mybir.dt.bfloat16
    fp16 = mybir.dt.float16
    u8 = mybir.dt.uint8
    Alu = mybir.AluOpType
    Act = mybir.ActivationFunctionType

    nc = bacc.Bacc(
        "TRN2",
        target_bir_lowering=False,
        debug=False,
        num_devices=NCORES,
    )
    xT = nc.dram_tensor("xT", [H, S], bf16, kind="ExternalInput")
    wp = nc.dram_tensor("wp", [PB, HB * 4], f32, kind="ExternalInput")
    wd0 = nc.dram_tensor("wd0", [PB, NPE * PB], bf16, kind="ExternalInput")
    wd = nc.dram_tensor("wd", [PB, (HB - 1) * NPE * PB], bf16,
                        kind="ExternalInput")
    yQ = nc.dram_tensor("yQ", [H, S], u8, kind="ExternalOutput")

    def is_bhalf(gidx):
        return B_PERIOD and (gidx % B_PERIOD) == B_PHASE

    with tile.TileContext(nc) as tc:
        with tc.tile_pool(name="wpool", bufs=1) as wpool, \
             tc.tile_pool(name="xpool", bufs=5) as xpool, \
             tc.tile_pool(name="ypool", bufs=3) as ypool, \
             tc.tile_pool(name="data", bufs=4) as pool, \
             tc.tile_pool(name="ppool", bufs=2, space="PSUM") as ppool:
            # wp columns per hb: 0 = w3*r, 1 = b*r + OFFSET, 2 = w0*r, 3 = pad
            wsb = wpool.tile([PB, HB * 4], f32)
            wdb0 = wpool.tile([PB, NPE * PB], bf16)
            wdb = wpool.tile([PB, (HB - 1) * NPE * PB], bf16)
            nc.scalar.dma_start(wdb0[:], wd0[:])
            nc.scalar.dma_start(wsb[:], wp[:])
            nc.scalar.dma_start(wdb[:], wd[:])
            # Tiny no-dep ACTIVATE so the ACT table load overlaps the first
            # x DMA instead of serializing in front of the first extraction.
            warm = wpool.tile([PB, 2], bf16)
            nc.vector.memset(warm[:], 0.0)
            nc.scalar.activation(warm[:], warm[:], Act.Identity, bias=0.0,
                                 scale=1.0)

            pend_store = []   # [(hb, y)] stores ~a block behind
            pend_conv = []    # [(y, s0, m)] B-half ACT converts, 1 half late

            def flush_conv():
                while pend_conv:
                    py, ps0, pm = pend_conv.pop(0)
                    nc.scalar.activation(py[:, ps0:ps0 + HW_], pm[:],
                                         Act.Copy, bias=0.0, scale=1.0)

            for hb in range(HB + 1):
                if hb < HB:
                    rows = slice(hb * PB, (hb + 1) * PB)
                    xt = xpool.tile([PB, PAD + S], bf16)
                    nc.gpsimd.memset(xt[:, 0:PAD], 0.0)
                    if hb == 0:
                        # first block in halves so compute starts early
                        nc.sync.dma_start(xt[:, PAD:PAD + HW_],
                                          xT[rows, 0:HW_])
                        nc.sync.dma_start(xt[:, PAD + HW_:PAD + S],
                                          xT[rows, HW_:S])
                    else:
                        nc.sync.dma_start(xt[:, PAD:PAD + S], xT[rows, :])
                    c = hb * 4
                    w3r = wsb[:, c + 0:c + 1]
                    bia = wsb[:, c + 1:c + 2]
                    w0r = wsb[:, c + 2:c + 3]
                    y = ypool.tile([PB, S], u8)
                    for half in range(S // HW_):
                        gidx = hb * 2 + half
                        s0 = half * HW_
                        base = PAD + s0
                        is_b = is_bhalf(gidx)
                        ps = ppool.tile([PB, HW_], f32)
                        ks = list(range(1, NPE) if is_b else range(NPE))
                        for k in ks:
                            if hb == 0:
                                dw = wdb0[:, k * PB:(k + 1) * PB]
                            else:
                                dcol = ((hb - 1) * NPE + k) * PB
                                dw = wdb[:, dcol:dcol + PB]
                            shift = base - (NPE - k)  # k=0 -> s-3 .. s-1
                            for bk in range(HW_ // BANK):
                                nc.tensor.matmul(
                                    ps[:, bk * BANK:(bk + 1) * BANK],
                                    dw,
                                    xt[:, shift + bk * BANK:
                                           shift + (bk + 1) * BANK],
                                    start=(k == ks[0]), stop=(k == ks[-1]),
                                    skip_group_check=True)
                        # ACT: e = ps + bias'  (fp16)
                        e = pool.tile([PB, HW_], fp16, tag="e", bufs=4)
                        nc.scalar.activation(e[:], ps[:], Act.Identity,
                                             bias=bia, scale=1.0)
                        # pending B-half converts go after the extraction
                        # they were waiting behind
                        flush_conv()
                        if is_b:
                            t03 = pool.tile([PB, HW_], fp16, tag="t03",
                                            bufs=2)
                            nc.vector.scalar_tensor_tensor(
                                t03[:], xt[:, base - 3:base - 3 + HW_], w0r,
                                e[:], op0=Alu.mult, op1=Alu.add)
                            m = pool.tile([PB, HW_], fp16, tag="m", bufs=2)
                            nc.vector.scalar_tensor_tensor(
                                m[:], xt[:, base:base + HW_], w3r, t03[:],
                                op0=Alu.mult, op1=Alu.add)
                            pend_conv.append((y, s0, m))
                        else:
                            ma = pool.tile([PB, HW_], fp16, tag="ma", bufs=3)
                            nc.vector.scalar_tensor_tensor(
                                ma[:], xt[:, base:base + HW_], w3r, e[:],
                                op0=Alu.mult, op1=Alu.add)
                            nc.vector.tensor_scalar(
                                y[:, s0:s0 + HW_], ma[:], 1.0, None,
                                op0=Alu.mult)
                        if half == 1 and pend_store:
                            phb, py = pend_store.pop(0)
                            prow = slice(phb * PB, (phb + 1) * PB)
                            nc.gpsimd.dma_start(yQ[prow, :], py[:])
                    pend_store.append((hb, y))
                else:
                    flush_conv()
                    phb, py = pend_store.pop()
                    prow = slice(phb * PB, (phb + 1) * PB)
                    nc.gpsimd.dma_start(yQ[prow, 0:HW_], py[:, 0:HW_])
                    nc.gpsimd.dma_start(yQ[prow, HW_:S], py[:, HW_:S])
    nc.compile()
    return nc


def get_nc():
    if "nc" not in _cached:
        _cached["nc"] = _build()
    return _cached["nc"]


def core_scales(weight, bias, xT_bf):
    """Per-channel quant scale r (H,) for one core from its bf16 x (H,S)."""
    xmax = np.abs(xT_bf.astype(np.float32)).max(axis=1)          # (H,)
    bound = np.abs(weight).sum(axis=0) * xmax + np.abs(bias)
    return (126.5 / bound).astype(np.float32)


def pack_weights(weight, bias, r):
    wp = np.empty((PB, HB * 4), dtype=np.float32)
    w3r = weight[3] * r
    br = bias * r + OFFSET
    w0r = weight[0] * r
    for hb in range(HB):
        sl = slice(hb * PB, (hb + 1) * PB)
        wp[:, hb * 4 + 0] = w3r[sl]
        wp[:, hb * 4 + 1] = br[sl]
        wp[:, hb * 4 + 2] = w0r[sl]
        wp[:, hb * 4 + 3] = 0.0
    return wp


def pack_diag(weight, r):
    """Per-block diag matrices for taps w0..w2 (scaled by r), bf16.

    Returns (wd0, wd): block 0 separately (loaded first, tiny) and blocks
    1..HB-1 concatenated.
    """
    import ml_dtypes
    wr = (weight[:NPE] * r[None, :]).astype(ml_dtypes.bfloat16)  # (NPE, H)
    wd_all = np.zeros((PB, HB * NPE * PB), dtype=ml_dtypes.bfloat16)
    idx = np.arange(PB)
    for hb in range(HB):
        for k in range(NPE):
            col = (hb * NPE + k) * PB
            wd_all[idx, col + idx] = wr[k, hb * PB + idx]
    return wd_all[:, :NPE * PB], wd_all[:, NPE * PB:]


def kernel(x, weight, bias):
    import ml_dtypes
    from concourse.bass_utils import run_bass_kernel_spmd

    x = np.asarray(x, dtype=np.float32)
    weight = np.asarray(weight, dtype=np.float32)
    bias = np.asarray(bias, dtype=np.float32)
    assert x.shape == (B, S, H), x.shape
    assert weight.shape == (KS, H), weight.shape
    assert bias.shape == (H,), bias.shape

    nc = get_nc()
    xT = x.transpose(0, 2, 1).astype(ml_dtypes.bfloat16)   # (B, H, S)
    rs, in_maps = [], []
    for i in range(NCORES):
        r = core_scales(weight, bias, xT[i])
        rs.append(r)
        wd0, wd = pack_diag(weight, r)
        in_maps.append({"xT": xT[i],
                        "wp": pack_weights(weight, bias, r),
                        "wd0": wd0, "wd": wd})
    try:
        res = run_bass_kernel_spmd(nc, in_maps, core_ids=list(range(NCORES)),
                                   **RUN_KWARGS)
    except Exception:
        res = run_bass_kernel_spmd(nc, in_maps, core_ids=list(range(NCORES)),
                                   **RUN_KWARGS)
    LAST_RESULTS.clear()
    LAST_RESULTS.append(res)
    out = np.empty((B, S, H), dtype=np.float32)
    for i in range(NCORES):
        u8v = res.results[i]["yQ"].astype(np.float32)      # (H, S)
        out[i] = ((u8v - 128.0) / rs[i][:, None]).T
    return out


# revision 5
# speedup vs baseline: 1.5497x; 1.5497x over previous
"""Causal depthwise conv1d (B=8, S=4096, H=2048, KS=4) on 8 trn2 NeuronCores.

Strategy (v3 — uint8 output wire, 3-way engine balance):
  - Shard batch across the 8 cores (one batch element each, no halo needed).
  - bf16 x on the wire (host casts + transposes to (H, S)): 16 MiB in/core.
  - uint8 y on the wire: the kernel emits u8 = rne(y*r_c + 128) with a
    per-channel scale r_c = 126.5 / (sum_k |w_k,c| * max_s |x_c,s| + |b_c|)
    computed on the host and folded into every device-side weight. The host
    dequantizes y = (u8 - 128)/r_c. 8 MiB out/core instead of 16. Measured
    end-to-end max-rel-err ~7e-3 vs the 2e-2 gate (deterministic inputs).
  - Per half-block (128 ch x 2048 cols), two variants batching the work:
    A-half (most):
      PE  : taps w0,w1,w2 as diag matmuls -> PSUM        12 MM ~2.62us
      ACT : e = ps + (b*r+128) -> fp16 (0.125 quantum)   ~1.97us
      DVE : m = x*w3r + e -> fp16 (stt, 2x mode)         ~1.27us
            y = u8(m)      (ts, 2x_2P mode)              ~1.13us
    B-half (every B_PERIOD-th, rebalances PE->ACT):
      PE  : taps w1,w2 only                              8 MM ~1.75us
      DVE : t03 = x(-3)*w0r + e -> fp16 ; m = x*w3r+t03  ~2.54us
      ACT : e as above PLUS y = u8(m) (convert emitted
            one half late to avoid FIFO head-block)      ~3.89us
    Totals ~ PE 79 / DVE 78 / ACT 75 / DMA ~70us.
  - Fill fix: block-0 diag weights live in a tiny separate dram tensor
    (98KB) so the first LDWEIGHTS doesn't wait for the full 1.5MB table.
  - Ring hygiene (from the bf16 baseline): x loads on the sync ring,
    stores + PAD memsets on gpsimd SWDGE, scalar ring = ACT only.
"""

import numpy as np

B, S, H, KS = 8, 4096, 2048, 4
NCORES = 8
PB = 128            # SBUF partitions
HB = H // PB        # 16 channel blocks per core
PAD = 4             # left zero-pad columns in the x tile (3 used + 1 align)
HW_ = 2048          # half-block width (PSUM tile = 4 banks)
BANK = 512          # PSUM bank width in f32 elements
NPE = 3             # taps held on PE (w0, w1, w2); w3 via the DVE stt

OFFSET = 128.0      # HW converts with round-to-nearest-even + saturate
B_PERIOD = 0        # every Nth half-block is a B-half (0 = none)
B_PHASE = 2         # gidx % B_PERIOD == B_PHASE -> B-half

RUN_KWARGS = {}
LAST_RESULTS = []

_cached = {}


def _build():
    import concourse.bacc as bacc
    import concourse.mybir as mybir
    import concourse.tile as tile

    f32 = mybir.dt.float32
    bf16 = mybir.dt.bfloat16
    fp16 = mybir.dt.float16
    u8 = mybir.dt.uint8
    Alu = mybir.AluOpType
    Act = mybir.ActivationFunctionType

    nc = bacc.Bacc(
        "TRN2",
        target_bir_lowering=False,
        debug=False,
        num_devices=NCORES,
    )
    xT = nc.dram_tensor("xT", [H, S], bf16, kind="ExternalInput")
    wp = nc.dram_tensor("wp", [PB, HB * 4], f32, kind="ExternalInput")
    wd0 = nc.dram_tensor("wd0", [PB, NPE * PB], bf16, kind="ExternalInput")
    wd = nc.dram_tensor("wd", [PB, (HB - 1) * NPE * PB], bf16,
                        kind="ExternalInput")
    yQ = nc.dram_tensor("yQ", [H, S], u8, kind="ExternalOutput")

    def is_bhalf(gidx):
        return B_PERIOD and (gidx % B_PERIOD) == B_PHASE

    with tile.TileContext(nc) as tc:
        with tc.tile_pool(name="wpool", bufs=1) as wpool, \
             tc.tile_pool(name="xpool", bufs=5) as xpool, \
             tc.tile_pool(name="ypool", bufs=3) as ypool, \
             tc.tile_pool(name="data", bufs=4) as pool, \
             tc.tile_pool(name="ppool", bufs=2, space="PSUM") as ppool:
            # wp columns per hb: 0 = w3*r, 1 = b*r + OFFSET, 2 = w0*r, 3 = pad
            wsb = wpool.tile([PB, HB * 4], f32)
            wdb0 = wpool.tile([PB, NPE * PB], bf16)
            wdb = wpool.tile([PB, (HB - 1) * NPE * PB], bf16)
            nc.scalar.dma_start(wdb0[:], wd0[:])
            nc.scalar.dma_start(wsb[:], wp[:])
            nc.scalar.dma_start(wdb[:], wd[:])
            # Tiny no-dep ACTIVATE so the ACT table load overlaps the first
            # x DMA instead of serializing in front of the first extraction.
            warm = wpool.tile([PB, 2], bf16)
            nc.vector.memset(warm[:], 0.0)
            nc.scalar.activation(warm[:], warm[:], Act.Identity, bias=0.0,
                                 scale=1.0)

            pend_store = []   # [(hb, y)] stores ~a block behind
            pend_conv = []    # [(y, s0, m)] B-half ACT converts, 1 half late

            def flush_conv():
                while pend_conv:
                    py, ps0, pm = pend_conv.pop(0)
                    nc.scalar.activation(py[:, ps0:ps0 + HW_], pm[:],
                                         Act.Copy, bias=0.0, scale=1.0)

            for hb in range(HB + 1):
                if hb < HB:
                    rows = slice(hb * PB, (hb + 1) * PB)
                    xt = xpool.tile([PB, PAD + S], bf16)
                    nc.gpsimd.memset(xt[:, 0:PAD], 0.0)
                    if hb == 0:
                        # first block in halves so compute starts early
                        nc.sync.dma_start(xt[:, PAD:PAD + HW_],
                                          xT[rows, 0:HW_])
                        nc.sync.dma_start(xt[:, PAD + HW_:PAD + S],
                                          xT[rows, HW_:S])
                    else:
                        nc.sync.dma_start(xt[:, PAD:PAD + S], xT[rows, :])
                    c = hb * 4
                    w3r = wsb[:, c + 0:c + 1]
                    bia = wsb[:, c + 1:c + 2]
                    w0r = wsb[:, c + 2:c + 3]
                    y = ypool.tile([PB, S], u8)
                    for half in range(S // HW_):
                        gidx = hb * 2 + half
                        s0 = half * HW_
                        base = PAD + s0
                        is_b = is_bhalf(gidx)
                        ps = ppool.tile([PB, HW_], f32)
                        ks = list(range(1, NPE) if is_b else range(NPE))
                        for k in ks:
                            if hb == 0:
                                dw = wdb0[:, k * PB:(k + 1) * PB]
                            else:
                                dcol = ((hb - 1) * NPE + k) * PB
                                dw = wdb[:, dcol:dcol + PB]
                            shift = base - (NPE - k)  # k=0 -> s-3 .. s-1
                            for bk in range(HW_ // BANK):
                                nc.tensor.matmul(
                                    ps[:, bk * BANK:(bk + 1) * BANK],
                                    dw,
                                    xt[:, shift + bk * BANK:
                                           shift + (bk + 1) * BANK],
                                    start=(k == ks[0]), stop=(k == ks[-1]),
                                    skip_group_check=True)
                        # ACT: e = ps + bias'  (fp16)
                        e = pool.tile([PB, HW_], fp16, tag="e", bufs=4)
                        nc.scalar.activation(e[:], ps[:], Act.Identity,
                                             bias=bia, scale=1.0)
                        # pending B-half converts go after the extraction
                        # they were waiting behind
                        flush_conv()
                        if is_b:
                            t03 = pool.tile([PB, HW_], fp16, tag="t03",
                                            bufs=2)
                            nc.vector.scalar_tensor_tensor(
                                t03[:], xt[:, base - 3:base - 3 + HW_], w0r,
                                e[:], op0=Alu.mult, op1=Alu.add)
                            m = pool.tile([PB, HW_], fp16, tag="m", bufs=2)
                            nc.vector.scalar_tensor_tensor(
                                m[:], xt[:, base:base + HW_], w3r, t03[:],
                                op0=Alu.mult, op1=Alu.add)
                            pend_conv.append((y, s0, m))
                        else:
                            nc.vector.scalar_tensor_tensor(
                                y[:, s0:s0 + HW_], xt[:, base:base + HW_],
                                w3r, e[:], op0=Alu.mult, op1=Alu.add)
                        if half == 1 and pend_store:
                            phb, py = pend_store.pop(0)
                            prow = slice(phb * PB, (phb + 1) * PB)
                            nc.gpsimd.dma_start(yQ[prow, :], py[:])
                    pend_store.append((hb, y))
                else:
                    flush_conv()
                    phb, py = pend_store.pop()
                    prow = slice(phb * PB, (phb + 1) * PB)
                    nc.gpsimd.dma_start(yQ[prow, 0:HW_], py[:, 0:HW_])
                    nc.gpsimd.dma_start(yQ[prow, HW_:S], py[:, HW_:S])
    nc.compile()
    return nc


def get_nc():
    if "nc" not in _cached:
        _cached["nc"] = _build()
    return _cached["nc"]


def core_scales(weight, bias, xT_bf):
    """Per-channel quant scale r (H,) for one core from its bf16 x (H,S)."""
    xmax = np.abs(xT_bf.astype(np.float32)).max(axis=1)          # (H,)
    bound = np.abs(weight).sum(axis=0) * xmax + np.abs(bias)
    return (126.5 / bound).astype(np.float32)


def pack_weights(weight, bias, r):
    wp = np.empty((PB, HB * 4), dtype=np.float32)
    w3r = weight[3] * r
    br = bias * r + OFFSET
    w0r = weight[0] * r
    for hb in range(HB):
        sl = slice(hb * PB, (hb + 1) * PB)
        wp[:, hb * 4 + 0] = w3r[sl]
        wp[:, hb * 4 + 1] = br[sl]
        wp[:, hb * 4 + 2] = w0r[sl]
        wp[:, hb * 4 + 3] = 0.0
    return wp


def pack_diag(weight, r):
    """Per-block diag matrices for taps w0..w2 (scaled by r), bf16.

    Returns (wd0, wd): block 0 separately (loaded first, tiny) and blocks
    1..HB-1 concatenated.
    """
    import ml_dtypes
    wr = (weight[:NPE] * r[None, :]).astype(ml_dtypes.bfloat16)  # (NPE, H)
    wd_all = np.zeros((PB, HB * NPE * PB), dtype=ml_dtypes.bfloat16)
    idx = np.arange(PB)
    for hb in range(HB):
        for k in range(NPE):
            col = (hb * NPE + k) * PB
            wd_all[idx, col + idx] = wr[k, hb * PB + idx]
    return wd_all[:, :NPE * PB], wd_all[:, NPE * PB:]


def kernel(x, weight, bias):
    import ml_dtypes
    from concourse.bass_utils import run_bass_kernel_spmd

    x = np.asarray(x, dtype=np.float32)
    weight = np.asarray(weight, dtype=np.float32)
    bias = np.asarray(bias, dtype=np.float32)
    assert x.shape == (B, S, H), x.shape
    assert weight.shape == (KS, H), weight.shape
    assert bias.shape == (H,), bias.shape

    nc = get_nc()
    xT = x.transpose(0, 2, 1).astype(ml_dtypes.bfloat16)   # (B, H, S)
    rs, in_maps = [], []
    for i in range(NCORES):
        r = core_scales(weight, bias, xT[i])
        rs.append(r)
        wd0, wd = pack_diag(weight, r)
        in_maps.append({"xT": xT[i],
                        "wp": pack_weights(weight, bias, r),
                        "wd0": wd0, "wd": wd})
    try:
        res = run_bass_kernel_spmd(nc, in_maps, core_ids=list(range(NCORES)),
                                   **RUN_KWARGS)
    except Exception:
        res = run_bass_kernel_spmd(nc, in_maps, core_ids=list(range(NCORES)),
                                   **RUN_KWARGS)
    LAST_RESULTS.clear()
    LAST_RESULTS.append(res)
    out = np.empty((B, S, H), dtype=np.float32)
    for i in range(NCORES):
        u8v = res.results[i]["yQ"].astype(np.float32)      # (H, S)
        out[i] = ((u8v - 128.0) / rs[i][:, None]).T
    return out
